# revision 5
# baseline (speedup 1.0000x reference)
"""CRF negative log-likelihood on 8 NeuronCores.

Strategy: the exp-space forward recurrence q_t = E_t * (A^T q_{t-1}) is a
product of positive matrices, which forgets its initial condition
geometrically (Hilbert-metric contraction).  So the time axis is cut into
C chunks per core; every chunk's chain starts W warmup steps early from a
uniform vector and runs independently — C parallel chains of K+W steps
instead of one 512-step serial chain.  Each chunk's result is correct up
to one unknown per-batch scale, which the host recovers by matching the
overlap point of consecutive chunks (a scalar log-ratio chain).

Per-step-growth is centred host-side (constants folded into A and E) so
no renormalisation is needed in fp32/bf16 range.  The gold-path score and
all log/selection work is done host-side in float64; the device only runs
the chains and records v[STOP] per step.
"""
import sys
import numpy as np
import ml_dtypes

sys.path.insert(0, "/opt/trn_rl_repo")

import concourse.bass as bass
import concourse.bacc as bacc
import concourse.mybir as mybir
import concourse.tile as tile
from concourse.bass_utils import run_bass_kernel_spmd

T, B, L = 512, 64, 48
START, STOP = L - 2, L - 1
NCORES = 8
BL = B // NCORES            # 8 batch rows per core

# chunking config
K = 16                      # steps owned per chunk
W = 16                      # warmup steps
S = K + W                   # chain length
C = T // K                  # chunks per core
CB = C * BL                 # total state columns per core (256)
NG = 4                      # chain groups (pipelined)
GC = C // NG                # chunks per group
GW = GC * BL                # state columns per group (64)
BLK = 8                     # steps per PSUM bank block
NB = (S + BLK - 1) // BLK   # blocks per chain

_FP = mybir.dt.float32
_BF = mybir.dt.bfloat16
BF = ml_dtypes.bfloat16
_cache = {}


def _build():
    nc = bacc.Bacc()
    ahat = nc.declare_dram_parameter("ahat", [L, L], _BF, isOutput=False)
    qinit = nc.declare_dram_parameter("qinit", [L, CB], _BF, isOutput=False)
    epk = nc.declare_dram_parameter("epk", [L, S * CB], _BF, isOutput=False)
    traj = nc.declare_dram_parameter("traj", [1, S * CB], _FP, isOutput=True)

    with tile.TileContext(nc) as tc:
        with (
            tc.tile_pool(name="consts", bufs=1) as consts,
            tc.tile_pool(name="state", bufs=1) as state,
            tc.tile_pool(name="ps", bufs=2, space="PSUM") as ps,
        ):
            ahat_sb = consts.tile([L, L], _BF)
            nc.gpsimd.dma_start(ahat_sb[:], ahat[:])

            eblk = []
            for sb in range(NB):
                e = consts.tile([L, BLK * CB], _BF, name=f"eblk{sb}")
                nc.gpsimd.dma_start(e[:], epk[:, sb * BLK * CB:(sb + 1) * BLK * CB])
                eblk.append(e)

            qa, qb = [], []
            for g in range(NG):
                q0 = state.tile([L, GW], _BF, name=f"qa{g}")
                nc.gpsimd.dma_start(q0[:], qinit[:, g * GW:(g + 1) * GW])
                q1 = state.tile([L, GW], _BF, name=f"qb{g}")
                qa.append(q0)
                qb.append(q1)

            traj_sb = state.tile([1, S * CB], _FP)

            vblk = [None] * NG
            for s in range(S):
                sb, si = divmod(s, BLK)
                for g in range(NG):
                    cur = qa[g] if s % 2 == 0 else qb[g]
                    nxt = qb[g] if s % 2 == 0 else qa[g]
                    if si == 0:
                        vblk[g] = ps.tile([L, BLK * GW], _FP, tag=f"v{g}",
                                          name=f"v{g}_{sb}")
                    vs = vblk[g][:, si * GW:(si + 1) * GW]
                    nc.tensor.matmul(vs, ahat_sb[:], cur[:])
                    if s < S - 1:
                        es = eblk[sb][:, si * CB + g * GW: si * CB + (g + 1) * GW]
                        nc.vector.tensor_mul(nxt[:], vs, es)
                    if si == BLK - 1:
                        # label axis is permuted host-side so STOP sits at
                        # partition 0 (Act can't start reads mid-partition)
                        dst = traj_sb[:, (g * NB + sb) * BLK * GW:
                                      (g * NB + sb + 1) * BLK * GW]
                        nc.scalar.copy(dst, vblk[g][0:1, :])

            nc.gpsimd.dma_start(traj[:], traj_sb[:])
    nc.finalize()
    return nc


def _get_nc():
    if "nc" not in _cache:
        _cache["nc"] = _build()
    return _cache["nc"]


def _logsumexp(x, axis):
    m = np.max(x, axis=axis, keepdims=True)
    return np.squeeze(m, axis) + np.log(np.sum(np.exp(x - m), axis=axis))


def kernel(feats, transitions, tags, mask):
    feats = np.asarray(feats, np.float64)
    transitions = np.asarray(transitions, np.float64)
    tags_in = np.asarray(tags)
    mask_in = np.asarray(mask)
    lengths = mask_in.sum(1).astype(np.int64)        # (B,)

    # ---- centring constants from a short host-side warmup ----
    part = feats[0] + transitions[START][None, :]    # (B, L)
    l0 = _logsumexp(part, 1)
    for t in range(1, 9):
        part = _logsumexp(part[:, :, None] + transitions[None, :, :], 1) + feats[t]
    g = float((_logsumexp(part, 1) - l0).mean()) / 8.0
    c2 = float(feats.mean())
    c1 = g - c2

    # ---- device operands (label axis permuted: swap 0 <-> STOP) ----
    P = np.arange(L)
    P[0], P[STOP] = STOP, 0
    Ahat = np.exp(transitions[P][:, P] - c1).astype(BF)   # (L, L) lhsT
    E = np.exp(np.float32(feats[:, :, P] - c2)).astype(BF)  # (T, B, L)

    t0 = np.array([0] + [c * K - W for c in range(1, C)])           # (C,)
    tidx = t0[:, None] + 1 + np.arange(S)[None, :]                  # (C, S)
    valid = tidx < T
    tclip = np.minimum(tidx, T - 1)
    # Epk_full[s, c, b, :]
    Epk = E[tclip].transpose(1, 0, 2, 3).copy()                     # (S, C, B, L)
    Epk[~valid.T] = BF(1.0)
    q0 = np.exp(np.float32(
        feats[0][:, P] + transitions[START][None, P] - 2 * c2))   # (B, L)

    in_maps = []
    for core in range(NCORES):
        bs = slice(BL * core, BL * (core + 1))
        ep = np.ascontiguousarray(
            Epk[:, :, bs, :].transpose(3, 0, 1, 2)).reshape(L, S * CB)
        qi = np.ones((L, C, BL), dtype=BF)
        qi[:, 0, :] = q0[bs].T.astype(BF)
        in_maps.append({
            "ahat": Ahat,
            "qinit": np.ascontiguousarray(qi.reshape(L, CB)),
            "epk": ep.astype(BF),
        })

    bkr = run_bass_kernel_spmd(_get_nc(), in_maps, list(range(NCORES)))
    global LAST_EXEC_NS
    LAST_EXEC_NS = bkr.exec_time_ns

    # ---- host-side stitch (float64) ----
    fwd = np.zeros(B)
    for core in range(NCORES):
        out = np.asarray(bkr.results[core]["traj"])
        # layout [g, sb, si, cg, b] -> [s, c, b]
        v = out.reshape(NG, NB, BLK, GC, BL).transpose(1, 2, 0, 3, 4)
        v = v.reshape(S, C, BL).astype(np.float64)
        lv = np.log(np.maximum(v, 1e-300))           # (S, C, BL)
        kap = np.zeros(C)
        kap[0] = 2 * c2

        def raw(c, t):   # log V_t for chunk c, all batches of this core
            return lv[t - t0[c] - 1, c] + c1 + (t - 1 - t0[c]) * g + kap[c]

        beta = np.zeros((C, BL))
        for c in range(1, C):
            tau = c * K
            beta[c] = beta[c - 1] + raw(c - 1, tau) - raw(c, tau)
        bs = slice(BL * core, BL * (core + 1))
        for j, b in enumerate(range(BL * core, BL * (core + 1))):
            l = int(lengths[b])
            c = (l - 1) // K
            fwd[b] = raw(c, l)[j] + beta[c, j]

    # ---- gold-path score (host, float64) ----
    tagsT = tags_in.T
    prev = np.concatenate([np.full((1, B), START, tags_in.dtype), tagsT[:-1]], 0)
    emit_sc = np.take_along_axis(feats, tagsT[:, :, None], axis=2)[..., 0]
    trans_sc = transitions[prev, tagsT]
    tg_energy = np.where(mask_in.T, emit_sc + trans_sc, 0.0).sum()
    end_ids = tags_in[np.arange(B), lengths - 1]
    gold = tg_energy + transitions[end_ids, STOP].sum()

    return np.float32(fwd.sum() - gold)


# revision 7
# speedup vs baseline: 1.2141x; 1.2141x over previous
"""CRF negative log-likelihood on 8 NeuronCores.

Strategy: the exp-space forward recurrence q_t = E_t * (A^T q_{t-1}) is a
product of positive matrices, which forgets its initial condition
geometrically (Hilbert-metric contraction).  So the time axis is cut into
C chunks per core; every chunk's chain starts W warmup steps early from a
uniform vector and runs independently — C parallel chains of K+W steps
instead of one 512-step serial chain.  Each chunk's result is correct up
to one unknown per-batch scale, which the host recovers by matching the
overlap point of consecutive chunks (a scalar log-ratio chain).

Per-step-growth is centred host-side (constants folded into A and E) so
no renormalisation is needed in fp32/bf16 range.  The gold-path score and
all log/selection work is done host-side in float64; the device only runs
the chains and records v[STOP] per step.
"""
import sys
import numpy as np
import ml_dtypes

sys.path.insert(0, "/opt/trn_rl_repo")

import concourse.bass as bass
import concourse.bacc as bacc
import concourse.mybir as mybir
import concourse.tile as tile
from concourse.bass_utils import run_bass_kernel_spmd

T, B, L = 512, 64, 48
START, STOP = L - 2, L - 1
NCORES = 8
BL = B // NCORES            # 8 batch rows per core

# chunking config
K = 16                      # steps owned per chunk
W = 12                      # warmup steps
S = K + W                   # chain length
C = T // K                  # chunks per core
CB = C * BL                 # total state columns per core (256)
NG = 2                      # chain groups (pipelined)
GC = C // NG                # chunks per group
GW = GC * BL                # state columns per group (128)
BLK = 4                     # steps per PSUM bank block
NB = (S + BLK - 1) // BLK   # blocks per chain

_FP = mybir.dt.float32
_BF = mybir.dt.bfloat16
BF = ml_dtypes.bfloat16
_cache = {}


def _build():
    nc = bacc.Bacc()
    ahat = nc.declare_dram_parameter("ahat", [L, L], _BF, isOutput=False)
    qinit = nc.declare_dram_parameter("qinit", [L, CB], _BF, isOutput=False)
    epk = nc.declare_dram_parameter("epk", [L, S * CB], _BF, isOutput=False)
    traj = nc.declare_dram_parameter("traj", [1, S * CB], _FP, isOutput=True)

    with tile.TileContext(nc) as tc:
        with (
            tc.tile_pool(name="consts", bufs=1) as consts,
            tc.tile_pool(name="state", bufs=1) as state,
            tc.tile_pool(name="ps", bufs=2, space="PSUM") as ps,
        ):
            # small DMAs that gate the first matmul go first, on the Sync
            # engine's queue so they are not stuck behind the big epk loads
            ahat_sb = consts.tile([L, L], _BF)
            nc.sync.dma_start(ahat_sb[:], ahat[:])

            qa, qb = [], []
            for g in range(NG):
                q0 = state.tile([L, GW], _BF, name=f"qa{g}")
                nc.sync.dma_start(q0[:], qinit[:, g * GW:(g + 1) * GW])
                q1 = state.tile([L, GW], _BF, name=f"qb{g}")
                qa.append(q0)
                qb.append(q1)

            eblk = []
            for sb in range(NB):
                e = consts.tile([L, BLK * CB], _BF, name=f"eblk{sb}")
                nc.gpsimd.dma_start(e[:], epk[:, sb * BLK * CB:(sb + 1) * BLK * CB])
                eblk.append(e)

            traj_sb = state.tile([1, S * CB], _FP)

            vblk = [None] * NG
            for s in range(S):
                sb, si = divmod(s, BLK)
                for g in range(NG):
                    cur = qa[g] if s % 2 == 0 else qb[g]
                    nxt = qb[g] if s % 2 == 0 else qa[g]
                    if si == 0:
                        vblk[g] = ps.tile([L, BLK * GW], _FP, tag=f"v{g}",
                                          name=f"v{g}_{sb}")
                    vs = vblk[g][:, si * GW:(si + 1) * GW]
                    nc.tensor.matmul(vs, ahat_sb[:], cur[:])
                    if s < S - 1:
                        es = eblk[sb][:, si * CB + g * GW: si * CB + (g + 1) * GW]
                        nc.vector.tensor_mul(nxt[:], vs, es)
                    if si == BLK - 1:
                        # label axis is permuted host-side so STOP sits at
                        # partition 0 (Act can't start reads mid-partition)
                        dst = traj_sb[:, (g * NB + sb) * BLK * GW:
                                      (g * NB + sb + 1) * BLK * GW]
                        nc.scalar.copy(dst, vblk[g][0:1, :])

            nc.gpsimd.dma_start(traj[:], traj_sb[:])
    nc.finalize()
    return nc


def _get_nc():
    if "nc" not in _cache:
        _cache["nc"] = _build()
    return _cache["nc"]


def _logsumexp(x, axis):
    m = np.max(x, axis=axis, keepdims=True)
    return np.squeeze(m, axis) + np.log(np.sum(np.exp(x - m), axis=axis))


def kernel(feats, transitions, tags, mask):
    feats = np.asarray(feats, np.float64)
    transitions = np.asarray(transitions, np.float64)
    tags_in = np.asarray(tags)
    mask_in = np.asarray(mask)
    lengths = mask_in.sum(1).astype(np.int64)        # (B,)

    # ---- centring constants from a short host-side warmup ----
    part = feats[0] + transitions[START][None, :]    # (B, L)
    l0 = _logsumexp(part, 1)
    for t in range(1, 9):
        part = _logsumexp(part[:, :, None] + transitions[None, :, :], 1) + feats[t]
    g = float((_logsumexp(part, 1) - l0).mean()) / 8.0
    c2 = float(feats.mean())
    c1 = g - c2

    # ---- device operands (label axis permuted: swap 0 <-> STOP) ----
    P = np.arange(L)
    P[0], P[STOP] = STOP, 0
    Ahat = np.exp(transitions[P][:, P] - c1).astype(BF)   # (L, L) lhsT
    E = np.exp(np.float32(feats[:, :, P] - c2)).astype(BF)  # (T, B, L)

    t0 = np.array([0] + [c * K - W for c in range(1, C)])           # (C,)
    tidx = t0[:, None] + 1 + np.arange(S)[None, :]                  # (C, S)
    valid = tidx < T
    tclip = np.minimum(tidx, T - 1)
    # Epk_full[s, c, b, :]
    Epk = E[tclip].transpose(1, 0, 2, 3).copy()                     # (S, C, B, L)
    Epk[~valid.T] = BF(1.0)
    q0 = np.exp(np.float32(
        feats[0][:, P] + transitions[START][None, P] - 2 * c2))   # (B, L)

    in_maps = []
    for core in range(NCORES):
        bs = slice(BL * core, BL * (core + 1))
        ep = np.ascontiguousarray(
            Epk[:, :, bs, :].transpose(3, 0, 1, 2)).reshape(L, S * CB)
        qi = np.ones((L, C, BL), dtype=BF)
        qi[:, 0, :] = q0[bs].T.astype(BF)
        in_maps.append({
            "ahat": Ahat,
            "qinit": np.ascontiguousarray(qi.reshape(L, CB)),
            "epk": ep.astype(BF),
        })

    bkr = run_bass_kernel_spmd(_get_nc(), in_maps, list(range(NCORES)))
    global LAST_EXEC_NS
    LAST_EXEC_NS = bkr.exec_time_ns

    # ---- host-side stitch (float64) ----
    fwd = np.zeros(B)
    for core in range(NCORES):
        out = np.asarray(bkr.results[core]["traj"])
        # layout [g, sb, si, cg, b] -> [s, c, b]
        v = out.reshape(NG, NB, BLK, GC, BL).transpose(1, 2, 0, 3, 4)
        v = v.reshape(S, C, BL).astype(np.float64)
        lv = np.log(np.maximum(v, 1e-300))           # (S, C, BL)
        kap = np.zeros(C)
        kap[0] = 2 * c2

        def raw(c, t):   # log V_t for chunk c, all batches of this core
            return lv[t - t0[c] - 1, c] + c1 + (t - 1 - t0[c]) * g + kap[c]

        beta = np.zeros((C, BL))
        for c in range(1, C):
            tau = c * K
            beta[c] = beta[c - 1] + raw(c - 1, tau) - raw(c, tau)
        bs = slice(BL * core, BL * (core + 1))
        for j, b in enumerate(range(BL * core, BL * (core + 1))):
            l = int(lengths[b])
            c = (l - 1) // K
            fwd[b] = raw(c, l)[j] + beta[c, j]

    # ---- gold-path score (host, float64) ----
    tagsT = tags_in.T
    prev = np.concatenate([np.full((1, B), START, tags_in.dtype), tagsT[:-1]], 0)
    emit_sc = np.take_along_axis(feats, tagsT[:, :, None], axis=2)[..., 0]
    trans_sc = transitions[prev, tagsT]
    tg_energy = np.where(mask_in.T, emit_sc + trans_sc, 0.0).sum()
    end_ids = tags_in[np.arange(B), lengths - 1]
    gold = tg_energy + transitions[end_ids, STOP].sum()

    return np.float32(fwd.sum() - gold)
